# revision 8
# baseline (speedup 1.0000x reference)
"""ColBERT late-interaction kernel for 8 TRN2 NeuronCores (Bass/Tile).

Problem (nn_ColBERT): B=64, LQ=32, LP=256, H=768, D=128.
  encode:  x = h @ W + b, normalized over the TOKEN axis (per (batch, d)).
  scores:  sims = einsum('bqd,cpd->bcqp', q, p); masked MaxSim over passage
           tokens, summed over query tokens -> [B, B] per side; output is
           concat([pos_scores, neg_scores], axis=1) -> [64, 128] fp32.

Sharding: passages (the dominant compute) are sharded across the 8 cores —
core j encodes+scores pos passages c in [8j, 8j+8) and neg passages in the
same range. Queries are replicated (cheap). Each core writes a disjoint
[64, 16] block of score columns; the host reassembles [64, 128]. No
collectives needed.

On-device layout: everything lives transposed as [D=128 partitions, tokens]
so the token-axis norm is a free-axis reduction, the per-(b,d) scale is a
per-partition scalar, and both late-interaction matmul operands are already
in [K=128, *] form. h^T tiles are produced by DMA xbar transpose (bf16).
Masking uses mask-multiply (masked token columns scaled to 0); a zero column
can only win the max if every real score in the row is negative, which has
probability ~2^-129 per row for continuous data.
"""

import numpy as np
import ml_dtypes

import concourse.bass as bass
from concourse import bacc
import concourse.mybir as mybir
import concourse.tile as tile
from concourse.bass_utils import run_bass_kernel_spmd

BF16 = ml_dtypes.bfloat16

B, LQ, LP, H, D = 64, 32, 256, 768, 128
NCORES = 8
CPC = B // NCORES            # passages per side per core (8)
NP_LOC = 2 * CPC             # local passages (pos+neg) = 16
PTOK = NP_LOC * LP           # 4096 passage tokens per core
QTOK = B * LQ                # 2048 query tokens (replicated)
KCH = H // 128               # 6 contraction chunks

F32 = mybir.dt.float32
BF = mybir.dt.bfloat16
AF = mybir.ActivationFunctionType
ALU = mybir.AluOpType
AX = mybir.AxisListType


def build_program(reps: int = 1, stop=None) -> bass.Bass:
    nc = bacc.Bacc(None)
    hq = nc.declare_dram_parameter("hq", [QTOK, H], BF, isOutput=False)
    hp = nc.declare_dram_parameter("hp", [PTOK, H], BF, isOutput=False)
    Wd = nc.declare_dram_parameter("W", [H, D], BF, isOutput=False)
    bd = nc.declare_dram_parameter("b", [D, 1], F32, isOutput=False)
    mk = nc.declare_dram_parameter("mask", [128, NP_LOC, LP], BF, isOutput=False)
    outd = nc.declare_dram_parameter("out", [B, NP_LOC], F32, isOutput=True)

    with tile.TileContext(nc) as tc:
        for _ in range(reps):
            _emit_body(nc, tc, hq, hp, Wd, bd, mk, outd, stop=stop)
    nc.finalize()
    return nc


def _emit_body(nc, tc, hq, hp, Wd, bd, mk, outd, dbg=None, stop=None):
    with (
        tc.tile_pool(name="const", bufs=1) as constp,
        tc.tile_pool(name="big", bufs=1) as bigp,
        tc.tile_pool(name="fin", bufs=1, space="PSUM") as finp,
    ):
        # ---- constants -------------------------------------------------
        Wt = constp.tile([128, KCH, 128], BF)          # W[k*128+p, d] at [p, k, d]
        nc.sync.dma_start(Wt[:], Wd.rearrange("(k p) d -> p k d", p=128))
        bcol = constp.tile([128, 1], F32)
        nc.sync.dma_start(bcol[:], bd[:])
        masksb = constp.tile([128, NP_LOC, LP], BF)    # host-replicated over partitions
        nc.sync.dma_start(masksb[:], mk[:])

        # encoded tokens, [d, token]: passages 0:PTOK, queries PTOK:PTOK+QTOK
        X = bigp.tile([128, PTOK + QTOK], BF)

        # ---- encode: X^T = W^T @ h^T + b -------------------------------
        with (
            tc.tile_pool(name="hT", bufs=2) as hTp,
            tc.tile_pool(name="xps", bufs=4, space="PSUM") as xpsp,
        ):
            for g in range(3):                          # 2048-token groups
                src, r0 = (hp, g * 2048) if g < 2 else (hq, 0)
                hT = hTp.tile([128, KCH, 2048], BF, tag="hT")
                for k in range(KCH):
                    nc.sync.dma_start(
                        hT[:, k, :], src[r0:r0 + 2048, k * 128:(k + 1) * 128],
                        transpose=True,
                    )
                for n in range(4):                      # 512-token psum chunks
                    ps = xpsp.tile([128, 512], F32, tag="x")
                    for k in range(KCH):
                        nc.tensor.matmul(
                            ps[:], Wt[:, k, :], hT[:, k, n * 512:(n + 1) * 512],
                            start=(k == 0), stop=(k == KCH - 1),
                        )
                    nc.scalar.activation(
                        X[:, g * 2048 + n * 512: g * 2048 + (n + 1) * 512],
                        ps[:], AF.Identity, bias=bcol[:, 0:1], scale=1.0,
                    )

        if dbg is not None:
            nc.sync.dma_start(dbg["dX"].rearrange("p t -> p t"), X[:])

        if stop == "encode":
            fsb = bigp.tile([B, NP_LOC], F32)
            nc.vector.tensor_copy(fsb[:], X[:B, :NP_LOC])
            nc.sync.dma_start(outd[:], fsb[:])
            return

        # ---- norms over token axis (per (b, d)) ------------------------
        # squares then segmented reduce_sum (vanilla ops only; the fused
        # tensor_tensor_reduce ANT op crashes the runtime on this path)
        nsqp = bigp.tile([128, NP_LOC], F32)
        nsqq = bigp.tile([128, B], F32)
        with tc.tile_pool(name="sqd", bufs=1) as sqdp:
            sq = sqdp.tile([128, PTOK + QTOK], BF)
            nc.vector.tensor_mul(sq[:, :PTOK], X[:, :PTOK], X[:, :PTOK])
            nc.vector.tensor_mul(sq[:, PTOK:], X[:, PTOK:], X[:, PTOK:])
            nc.vector.reduce_sum(
                nsqp[:], sq[:, :PTOK].rearrange("p (c l) -> p c l", c=NP_LOC),
                axis=AX.X)
            nc.vector.reduce_sum(
                nsqq[:], sq[:, PTOK:].rearrange("p (b l) -> p b l", b=B),
                axis=AX.X)

        if dbg is not None:
            nc.sync.dma_start(dbg["dnsqp"][:], nsqp[:])

        if stop == "norms":
            fsb = bigp.tile([B, NP_LOC], F32)
            nc.vector.tensor_copy(fsb[:], nsqp[:B, :])
            nc.sync.dma_start(outd[:], fsb[:])
            return

        # scale = 1 / max(sqrt(nsq), 1e-12)
        sclp = bigp.tile([128, NP_LOC], F32)
        sclq = bigp.tile([128, B], F32)
        nc.scalar.sqrt(sclp[:], nsqp[:])
        nc.vector.tensor_scalar_max(sclp[:], sclp[:], 1e-12)
        nc.vector.reciprocal(sclp[:], sclp[:])
        nc.scalar.sqrt(sclq[:], nsqq[:])
        nc.vector.tensor_scalar_max(sclq[:], sclq[:], 1e-12)
        nc.vector.reciprocal(sclq[:], sclq[:])

        # ---- scale (+ mask on the passage side) ------------------------
        P2 = bigp.tile([128, PTOK], BF)
        Q2 = bigp.tile([128, QTOK], BF)
        for c in range(NP_LOC):
            nc.vector.scalar_tensor_tensor(
                out=P2[:, c * LP:(c + 1) * LP],
                in0=X[:, c * LP:(c + 1) * LP],
                scalar=sclp[:, c:c + 1],
                in1=masksb[:, c, :],
                op0=ALU.mult, op1=ALU.mult,
            )
        for b_ in range(B):
            nc.vector.tensor_scalar_mul(
                Q2[:, b_ * LQ:(b_ + 1) * LQ],
                X[:, PTOK + b_ * LQ:PTOK + (b_ + 1) * LQ],
                sclq[:, b_:b_ + 1],
            )

        if dbg is not None:
            nc.sync.dma_start(dbg["dP2"][:], P2[:])
            nc.sync.dma_start(dbg["dQ2"][:], Q2[:])

        if stop == "scale":
            fsb = bigp.tile([B, NP_LOC], F32)
            nc.vector.tensor_copy(fsb[:], P2[:B, :NP_LOC])
            nc.sync.dma_start(outd[:], fsb[:])
            return

        # ---- scores: per query-tile, matmul against all passages, ------
        # ---- masked MaxSim via free-axis reduce_max --------------------
        NQT = QTOK // 128                               # 16 query tiles
        Mx = bigp.tile([128, NQT * NP_LOC], F32)        # [q, qt*16+c] maxima
        CGROUPS = (6, 6, 4)
        with tc.tile_pool(name="sps", bufs=2, space="PSUM") as spsp:
            for qt in range(NQT):
                c0 = 0
                for gsz in CGROUPS:
                    ps = spsp.tile([128, 6, LP], F32, tag="s")
                    for i in range(gsz):
                        c = c0 + i
                        nc.tensor.matmul(
                            ps[:, i, :],
                            Q2[:, qt * 128:(qt + 1) * 128],
                            P2[:, c * LP:(c + 1) * LP],
                            start=True, stop=True,
                        )
                    nc.vector.reduce_max(
                        Mx[:, qt * NP_LOC + c0: qt * NP_LOC + c0 + gsz],
                        ps[:, :gsz, :], axis=AX.X,
                    )
                    c0 += gsz

        if dbg is not None:
            nc.sync.dma_start(dbg["dMx"][:], Mx[:])

        if stop == "maxred":
            fsb = bigp.tile([B, NP_LOC], F32)
            nc.vector.tensor_copy(fsb[:], Mx[:B, :NP_LOC])
            nc.sync.dma_start(outd[:], fsb[:])
            return

        # ---- sum over the 32 queries of each batch via PE --------------
        A = bigp.tile([128, 4], F32)                    # block-ones
        nc.vector.memset(A[:], 0.0)
        for i in range(4):
            nc.vector.memset(A[32 * i:32 * (i + 1), i:i + 1], 1.0)
        fin = finp.tile([4, NQT * NP_LOC], F32)
        for qt in range(NQT):
            nc.tensor.matmul(
                fin[:, qt * NP_LOC:(qt + 1) * NP_LOC],
                A[:], Mx[:, qt * NP_LOC:(qt + 1) * NP_LOC],
                start=True, stop=True,
            )
        # fin[b4, qt*16+c] -> out[qt*4+b4, c]
        fsb = bigp.tile([4, NQT * NP_LOC], F32)
        nc.vector.tensor_copy(fsb[:], fin[:])
        nc.sync.dma_start(
            outd.rearrange("(t b4) c -> b4 t c", b4=4),
            fsb[:].rearrange("p (t c) -> p t c", t=NQT),
        )


def _prep_inputs(query_hidden, positive_hidden, negative_hidden, W, b,
                 positive_filter_mask, negative_filter_mask):
    hq = np.ascontiguousarray(
        np.asarray(query_hidden, np.float32).reshape(QTOK, H)).astype(BF16)
    Wb = np.asarray(W, np.float32).astype(BF16)
    bcol = np.ascontiguousarray(np.asarray(b, np.float32).reshape(D, 1))
    pos = np.asarray(positive_hidden, np.float32)
    neg = np.asarray(negative_hidden, np.float32)
    pmask = np.asarray(positive_filter_mask)
    nmask = np.asarray(negative_filter_mask)

    in_maps = []
    for j in range(NCORES):
        sl = slice(j * CPC, (j + 1) * CPC)
        hp = np.concatenate(
            [pos[sl].reshape(CPC * LP, H), neg[sl].reshape(CPC * LP, H)], axis=0
        ).astype(BF16)
        m = np.concatenate([pmask[sl], nmask[sl]], axis=0).astype(np.float32)
        mrep = np.ascontiguousarray(
            np.broadcast_to(m[None, :, :], (128, NP_LOC, LP))).astype(BF16)
        in_maps.append({
            "hq": hq, "hp": hp, "W": Wb, "b": bcol, "mask": mrep,
        })
    return in_maps


def _assemble(results):
    out = np.zeros((B, 2 * B), np.float32)
    for j in range(NCORES):
        o = np.asarray(results[j]["out"], np.float32)      # [64, 16]
        out[:, j * CPC:(j + 1) * CPC] = o[:, :CPC]
        out[:, B + j * CPC:B + (j + 1) * CPC] = o[:, CPC:]
    return out


def kernel(query_hidden, positive_hidden, negative_hidden, W, b,
           positive_filter_mask, negative_filter_mask):
    in_maps = _prep_inputs(query_hidden, positive_hidden, negative_hidden,
                           W, b, positive_filter_mask, negative_filter_mask)
    nc = build_program()
    res = run_bass_kernel_spmd(nc, in_maps, list(range(NCORES)))
    return _assemble(res.results)


# revision 10
# speedup vs baseline: 95.6539x; 95.6539x over previous
"""ColBERT late-interaction kernel for 8 TRN2 NeuronCores (Bass/Tile).

Problem (nn_ColBERT): B=64, LQ=32, LP=256, H=768, D=128.
  encode:  x = h @ W + b, normalized over the TOKEN axis (per (batch, d)).
  scores:  sims = einsum('bqd,cpd->bcqp', q, p); masked MaxSim over passage
           tokens, summed over query tokens -> [B, B] per side; output is
           concat([pos_scores, neg_scores], axis=1) -> [64, 128] fp32.

Sharding: passages (the dominant compute) are sharded across the 8 cores —
core j handles pos passages c in [8j, 8j+8) and neg passages in the same
range. Queries are replicated (cheap). Each core writes a disjoint [64, 16]
block of score columns; the host reassembles [64, 128]. No collectives.

On-device layout: everything lives transposed as [D=128 partitions, tokens]
so the token-axis norm is a free-axis reduction, the per-(b,d) scale is a
per-partition scalar, and both late-interaction matmul operands are already
in [K=128, *] form. h^T tiles are produced by DMA xbar transpose (bf16).

Masking via compaction: the filter masks are known on the host when the
program is built, so each passage's unmasked token positions are shipped as
an index tensor and a second, compacted h^T is gathered directly from DRAM
with dma_gather(transpose=True) (padded to n_pad with the passage's token 0,
which the problem guarantees unmasked — duplicates never change a max).
The compacted tokens are encoded again (cheap on the PE) and the MaxSim
matmuls + DVE reduce_max then touch only ~n_pad of 256 columns per passage,
which is what the whole kernel is bottlenecked on. Norms still come from the
full encode, faithful to the reference (masked tokens do contribute there).
"""

import numpy as np
import ml_dtypes

import concourse.bass as bass
from concourse import bacc
import concourse.mybir as mybir
import concourse.tile as tile
from concourse.bass_utils import run_bass_kernel_spmd

BF16 = ml_dtypes.bfloat16

B, LQ, LP, H, D = 64, 32, 256, 768, 128
NCORES = 8
CPC = B // NCORES            # passages per side per core (8)
NP_LOC = 2 * CPC             # local passages (pos+neg) = 16
PTOK = NP_LOC * LP           # 4096 passage tokens per core
QTOK = B * LQ                # 2048 query tokens (replicated)
KCH = H // 128               # 6 contraction chunks

F32 = mybir.dt.float32
BF = mybir.dt.bfloat16
I16 = mybir.dt.int16
AF = mybir.ActivationFunctionType
ALU = mybir.AluOpType
AX = mybir.AxisListType


def build_program(n_pad: int, reps: int = 1, stop=None) -> bass.Bass:
    ni = NP_LOC * n_pad
    nc = bacc.Bacc(None)
    hq = nc.declare_dram_parameter("hq", [QTOK, H], BF, isOutput=False)
    hp = nc.declare_dram_parameter("hp", [PTOK, H], BF, isOutput=False)
    Wd = nc.declare_dram_parameter("W", [H, D], BF, isOutput=False)
    bd = nc.declare_dram_parameter("b", [D, 1], F32, isOutput=False)
    pidx = nc.declare_dram_parameter("pidx", [128, ni // 16], I16, isOutput=False)
    outd = nc.declare_dram_parameter("out", [B, NP_LOC], F32, isOutput=True)

    with tile.TileContext(nc) as tc:
        for _ in range(reps):
            _emit_body(nc, tc, hq, hp, Wd, bd, pidx, outd, n_pad, stop=stop)
    nc.finalize()
    return nc


def _emit_body(nc, tc, hq, hp, Wd, bd, pidx, outd, n_pad, stop=None):
    ni = NP_LOC * n_pad
    with (
        tc.tile_pool(name="const", bufs=1) as constp,
        tc.tile_pool(name="big", bufs=1) as bigp,
        tc.tile_pool(name="fin", bufs=1, space="PSUM") as finp,
    ):
        # ---- constants -------------------------------------------------
        Wt = constp.tile([128, KCH, 128], BF)          # W[k*128+p, d] at [p, k, d]
        nc.sync.dma_start(Wt[:], Wd.rearrange("(k p) d -> p k d", p=128))
        bcol = constp.tile([128, 1], F32)
        nc.sync.dma_start(bcol[:], bd[:])
        idxt = constp.tile([128, ni // 16], I16)
        nc.sync.dma_start(idxt[:], pidx[:])

        # encoded tokens, [d, token]: passages 0:PTOK, queries PTOK:PTOK+QTOK
        X = bigp.tile([128, PTOK + QTOK], BF)
        # compacted-passage encodings
        Xc = bigp.tile([128, ni], BF)

        # ---- encode: X^T = W^T @ h^T + b -------------------------------
        with (
            tc.tile_pool(name="hT", bufs=2) as hTp,
            tc.tile_pool(name="xps", bufs=4, space="PSUM") as xpsp,
        ):
            for g in range(3):                          # 2048-token groups
                src, r0 = (hp, g * 2048) if g < 2 else (hq, 0)
                hT = hTp.tile([128, KCH, 2048], BF, tag="hT")
                for k in range(KCH):
                    nc.sync.dma_start(
                        hT[:, k, :], src[r0:r0 + 2048, k * 128:(k + 1) * 128],
                        transpose=True,
                    )
                for n in range(4):                      # 512-token psum chunks
                    ps = xpsp.tile([128, 512], F32, tag="x")
                    for k in range(KCH):
                        nc.tensor.matmul(
                            ps[:], Wt[:, k, :], hT[:, k, n * 512:(n + 1) * 512],
                            start=(k == 0), stop=(k == KCH - 1),
                        )
                    nc.scalar.activation(
                        X[:, g * 2048 + n * 512: g * 2048 + (n + 1) * 512],
                        ps[:], AF.Identity, bias=bcol[:, 0:1], scale=1.0,
                    )

            # compacted h^T gathered straight from DRAM (tokens = unmasked
            # positions per passage, padded with each passage's token 0),
            # chunk-major so each gather's output is contiguous
            nch = ni // 512
            hTc = hTp.tile([128, nch, KCH, 512], BF, tag="hTc")
            for ch in range(nch):
                o = ch * 512
                nc.gpsimd.dma_gather(
                    out_ap=hTc[:, ch], in_ap=hp[:],
                    idxs_ap=idxt[:, o // 16:(o + 512) // 16],
                    num_idxs=512, num_idxs_reg=512, elem_size=H, transpose=True,
                )
            for ch in range(nch):
                ps = xpsp.tile([128, 512], F32, tag="x")
                for k in range(KCH):
                    nc.tensor.matmul(
                        ps[:], Wt[:, k, :], hTc[:, ch, k, :],
                        start=(k == 0), stop=(k == KCH - 1),
                    )
                nc.scalar.activation(
                    Xc[:, ch * 512:(ch + 1) * 512], ps[:],
                    AF.Identity, bias=bcol[:, 0:1], scale=1.0,
                )

        if stop == "encode":
            fsb = bigp.tile([B, NP_LOC], F32)
            nc.vector.tensor_copy(fsb[:], X[:B, :NP_LOC])
            nc.sync.dma_start(outd[:], fsb[:])
            return

        # ---- norms over token axis (per (b, d)), from the FULL encode --
        nsqp = bigp.tile([128, NP_LOC], F32)
        nsqq = bigp.tile([128, B], F32)
        with tc.tile_pool(name="sqd", bufs=1) as sqdp:
            sq = sqdp.tile([128, PTOK + QTOK], BF)
            nc.vector.tensor_mul(sq[:, :PTOK], X[:, :PTOK], X[:, :PTOK])
            nc.vector.tensor_mul(sq[:, PTOK:], X[:, PTOK:], X[:, PTOK:])
            nc.vector.reduce_sum(
                nsqp[:], sq[:, :PTOK].rearrange("p (c l) -> p c l", c=NP_LOC),
                axis=AX.X)
            nc.vector.reduce_sum(
                nsqq[:], sq[:, PTOK:].rearrange("p (b l) -> p b l", b=B),
                axis=AX.X)

        if stop == "norms":
            fsb = bigp.tile([B, NP_LOC], F32)
            nc.vector.tensor_copy(fsb[:], nsqp[:B, :])
            nc.sync.dma_start(outd[:], fsb[:])
            return

        # scale = 1 / max(sqrt(nsq), 1e-12)
        sclp = bigp.tile([128, NP_LOC], F32)
        sclq = bigp.tile([128, B], F32)
        nc.scalar.sqrt(sclp[:], nsqp[:])
        nc.vector.tensor_scalar_max(sclp[:], sclp[:], 1e-12)
        nc.vector.reciprocal(sclp[:], sclp[:])
        nc.scalar.sqrt(sclq[:], nsqq[:])
        nc.vector.tensor_scalar_max(sclq[:], sclq[:], 1e-12)
        nc.vector.reciprocal(sclq[:], sclq[:])

        # ---- scale (compacted passages need no mask) -------------------
        P2 = bigp.tile([128, ni], BF)
        Q2 = bigp.tile([128, QTOK], BF)
        for c in range(NP_LOC):
            nc.vector.tensor_scalar_mul(
                P2[:, c * n_pad:(c + 1) * n_pad],
                Xc[:, c * n_pad:(c + 1) * n_pad],
                sclp[:, c:c + 1],
            )
        for b_ in range(B):
            nc.vector.tensor_scalar_mul(
                Q2[:, b_ * LQ:(b_ + 1) * LQ],
                X[:, PTOK + b_ * LQ:PTOK + (b_ + 1) * LQ],
                sclq[:, b_:b_ + 1],
            )

        if stop == "scale":
            fsb = bigp.tile([B, NP_LOC], F32)
            nc.vector.tensor_copy(fsb[:], P2[:B, :NP_LOC])
            nc.sync.dma_start(outd[:], fsb[:])
            return

        # ---- scores: per query-tile, matmul against compacted passages,
        # ---- MaxSim via free-axis reduce_max ---------------------------
        # psum tiles keep a 256-column (1KB, bank-aligned) stride per
        # passage; only the first n_pad columns are written and reduced.
        NQT = QTOK // 128                               # 16 query tiles
        Mx = bigp.tile([128, NQT * NP_LOC], F32)        # [q, qt*16+c] maxima
        CGROUPS = (6, 6, 4)
        with tc.tile_pool(name="sps", bufs=2, space="PSUM") as spsp:
            for qt in range(NQT):
                c0 = 0
                for gsz in CGROUPS:
                    ps = spsp.tile([128, 6, LP], F32, tag="s")
                    for i in range(gsz):
                        c = c0 + i
                        nc.tensor.matmul(
                            ps[:, i, :n_pad],
                            Q2[:, qt * 128:(qt + 1) * 128],
                            P2[:, c * n_pad:(c + 1) * n_pad],
                            start=True, stop=True,
                        )
                    nc.vector.reduce_max(
                        Mx[:, qt * NP_LOC + c0: qt * NP_LOC + c0 + gsz],
                        ps[:, :gsz, :n_pad], axis=AX.X,
                    )
                    c0 += gsz

        if stop == "maxred":
            fsb = bigp.tile([B, NP_LOC], F32)
            nc.vector.tensor_copy(fsb[:], Mx[:B, :NP_LOC])
            nc.sync.dma_start(outd[:], fsb[:])
            return

        # ---- sum over the 32 queries of each batch via PE --------------
        A = bigp.tile([128, 4], F32)                    # block-ones
        nc.vector.memset(A[:], 0.0)
        for i in range(4):
            nc.vector.memset(A[32 * i:32 * (i + 1), i:i + 1], 1.0)
        fin = finp.tile([4, NQT * NP_LOC], F32)
        for qt in range(NQT):
            nc.tensor.matmul(
                fin[:, qt * NP_LOC:(qt + 1) * NP_LOC],
                A[:], Mx[:, qt * NP_LOC:(qt + 1) * NP_LOC],
                start=True, stop=True,
            )
        # fin[b4, qt*16+c] -> out[qt*4+b4, c]
        fsb = bigp.tile([4, NQT * NP_LOC], F32)
        nc.vector.tensor_copy(fsb[:], fin[:])
        nc.sync.dma_start(
            outd.rearrange("(t b4) c -> b4 t c", b4=4),
            fsb[:].rearrange("p (t c) -> p t c", t=NQT),
        )


def _prep_inputs(query_hidden, positive_hidden, negative_hidden, W, b,
                 positive_filter_mask, negative_filter_mask):
    """Returns (in_maps, n_pad)."""
    hq = np.ascontiguousarray(
        np.asarray(query_hidden, np.float32).reshape(QTOK, H)).astype(BF16)
    Wb = np.asarray(W, np.float32).astype(BF16)
    bcol = np.ascontiguousarray(np.asarray(b, np.float32).reshape(D, 1))
    pos = np.asarray(positive_hidden, np.float32)
    neg = np.asarray(negative_hidden, np.float32)
    pmask = np.asarray(positive_filter_mask).astype(bool)
    nmask = np.asarray(negative_filter_mask).astype(bool)

    core_masks = []
    n_max = 1
    for j in range(NCORES):
        sl = slice(j * CPC, (j + 1) * CPC)
        m = np.concatenate([pmask[sl], nmask[sl]], axis=0)   # [16, 256]
        core_masks.append(m)
        n_max = max(n_max, int(m.sum(axis=1).max()))
    n_pad = -(-n_max // 32) * 32          # mult of 32 so ni % 512 == 0
    ni = NP_LOC * n_pad

    in_maps = []
    for j in range(NCORES):
        sl = slice(j * CPC, (j + 1) * CPC)
        hp = np.concatenate(
            [pos[sl].reshape(CPC * LP, H), neg[sl].reshape(CPC * LP, H)], axis=0
        ).astype(BF16)
        m = core_masks[j]
        idx = np.zeros((NP_LOC, n_pad), np.int16)
        for c in range(NP_LOC):
            pos_c = np.nonzero(m[c])[0]
            idx[c, :len(pos_c)] = pos_c + c * LP
            idx[c, len(pos_c):] = c * LP                      # dup token 0
        flat = idx.reshape(ni)
        wrapped = np.tile(flat.reshape(ni // 16, 16).T, (8, 1)).copy()
        in_maps.append({
            "hq": hq, "hp": hp, "W": Wb, "b": bcol, "pidx": wrapped,
        })
    return in_maps, n_pad


def _assemble(results):
    out = np.zeros((B, 2 * B), np.float32)
    for j in range(NCORES):
        o = np.asarray(results[j]["out"], np.float32)      # [64, 16]
        out[:, j * CPC:(j + 1) * CPC] = o[:, :CPC]
        out[:, B + j * CPC:B + (j + 1) * CPC] = o[:, CPC:]
    return out


def kernel(query_hidden, positive_hidden, negative_hidden, W, b,
           positive_filter_mask, negative_filter_mask):
    in_maps, n_pad = _prep_inputs(query_hidden, positive_hidden, negative_hidden,
                                  W, b, positive_filter_mask, negative_filter_mask)
    nc = build_program(n_pad)
    res = run_bass_kernel_spmd(nc, in_maps, list(range(NCORES)))
    return _assemble(res.results)


# revision 11
# speedup vs baseline: 281.7352x; 2.9454x over previous
"""ColBERT late-interaction kernel for 8 TRN2 NeuronCores (Bass/Tile).

Problem (nn_ColBERT): B=64, LQ=32, LP=256, H=768, D=128.
  encode:  x = h @ W + b, normalized over the TOKEN axis (per (batch, d)).
  scores:  sims = einsum('bqd,cpd->bcqp', q, p); masked MaxSim over passage
           tokens, summed over query tokens -> [B, B] per side; output is
           concat([pos_scores, neg_scores], axis=1) -> [64, 128] fp32.

Sharding: passages (the dominant compute) are sharded across the 8 cores —
core j handles pos passages c in [8j, 8j+8) and neg passages in the same
range. Queries are replicated (cheap). Each core writes a disjoint [64, 16]
block of score columns; the host reassembles [64, 128]. No collectives.

On-device layout: everything lives transposed as [D=128 partitions, tokens]
so the token-axis norm is a free-axis reduction, the per-(b,d) scale is a
per-partition scalar, and both late-interaction matmul operands are already
in [K=128, *] form. h^T tiles are produced by DMA xbar transpose (bf16).

Masking via compaction: the filter masks are known on the host when the
program is built, so each passage's unmasked token positions are shipped as
an index tensor and a second, compacted h^T is gathered directly from DRAM
with dma_gather(transpose=True) (padded to n_pad with the passage's token 0,
which the problem guarantees unmasked — duplicates never change a max).
The compacted tokens are encoded again (cheap on the PE) and the MaxSim
matmuls + DVE reduce_max then touch only ~n_pad of 256 columns per passage,
which is what the whole kernel is bottlenecked on. Norms still come from the
full encode, faithful to the reference (masked tokens do contribute there).
"""

import numpy as np
import ml_dtypes

import concourse.bass as bass
from concourse import bacc
import concourse.mybir as mybir
import concourse.tile as tile
from concourse.bass_utils import run_bass_kernel_spmd

BF16 = ml_dtypes.bfloat16

B, LQ, LP, H, D = 64, 32, 256, 768, 128
NCORES = 8
CPC = B // NCORES            # passages per side per core (8)
NP_LOC = 2 * CPC             # local passages (pos+neg) = 16
PTOK = NP_LOC * LP           # 4096 passage tokens per core
QTOK = B * LQ                # 2048 query tokens (replicated)
KCH = H // 128               # 6 contraction chunks

F32 = mybir.dt.float32
BF = mybir.dt.bfloat16
I16 = mybir.dt.int16
AF = mybir.ActivationFunctionType
ALU = mybir.AluOpType
AX = mybir.AxisListType


def build_program(n_pad: int, reps: int = 1, stop=None) -> bass.Bass:
    ni = NP_LOC * n_pad
    nc = bacc.Bacc(None)
    hq = nc.declare_dram_parameter("hq", [QTOK, H], BF, isOutput=False)
    hp = nc.declare_dram_parameter("hp", [PTOK, H], BF, isOutput=False)
    Wd = nc.declare_dram_parameter("W", [H, D], BF, isOutput=False)
    bd = nc.declare_dram_parameter("b", [D, 1], F32, isOutput=False)
    pidx = nc.declare_dram_parameter("pidx", [128, ni // 16], I16, isOutput=False)
    outd = nc.declare_dram_parameter("out", [B, NP_LOC], F32, isOutput=True)

    with tile.TileContext(nc) as tc:
        for _ in range(reps):
            _emit_body(nc, tc, hq, hp, Wd, bd, pidx, outd, n_pad, stop=stop)
    nc.finalize()
    return nc


def _emit_body(nc, tc, hq, hp, Wd, bd, pidx, outd, n_pad, stop=None):
    ni = NP_LOC * n_pad
    with (
        tc.tile_pool(name="const", bufs=1) as constp,
        tc.tile_pool(name="big", bufs=1) as bigp,
        tc.tile_pool(name="fin", bufs=1, space="PSUM") as finp,
    ):
        # ---- constants -------------------------------------------------
        Wt = constp.tile([128, KCH, 128], BF)          # W[k*128+p, d] at [p, k, d]
        nc.sync.dma_start(Wt[:], Wd.rearrange("(k p) d -> p k d", p=128))
        bcol = constp.tile([128, 1], F32)
        nc.sync.dma_start(bcol[:], bd[:])
        idxt = constp.tile([128, ni // 16], I16)
        nc.sync.dma_start(idxt[:], pidx[:])

        # encoded tokens, [d, token]: passages 0:PTOK, queries PTOK:PTOK+QTOK
        X = bigp.tile([128, PTOK + QTOK], BF)
        # compacted-passage encodings
        Xc = bigp.tile([128, ni], BF)

        # ---- encode: X^T = W^T @ h^T + b -------------------------------
        with (
            tc.tile_pool(name="hT", bufs=2) as hTp,
            tc.tile_pool(name="xps", bufs=4, space="PSUM") as xpsp,
        ):
            for g in range(3):                          # 2048-token groups
                src, r0 = (hp, g * 2048) if g < 2 else (hq, 0)
                hT = hTp.tile([128, KCH, 2048], BF, tag="hT")
                for k in range(KCH):
                    nc.sync.dma_start(
                        hT[:, k, :], src[r0:r0 + 2048, k * 128:(k + 1) * 128],
                        transpose=True,
                    )
                for n in range(4):                      # 512-token psum chunks
                    ps = xpsp.tile([128, 512], F32, tag="x")
                    for k in range(KCH):
                        nc.tensor.matmul(
                            ps[:], Wt[:, k, :], hT[:, k, n * 512:(n + 1) * 512],
                            start=(k == 0), stop=(k == KCH - 1),
                        )
                    nc.scalar.activation(
                        X[:, g * 2048 + n * 512: g * 2048 + (n + 1) * 512],
                        ps[:], AF.Identity, bias=bcol[:, 0:1], scale=1.0,
                    )

            # compacted h^T gathered straight from DRAM (tokens = unmasked
            # positions per passage, padded with each passage's token 0),
            # chunk-major so each gather's output is contiguous
            nch = ni // 512
            hTc = hTp.tile([128, nch, KCH, 512], BF, tag="hTc")
            for ch in range(nch):
                o = ch * 512
                nc.gpsimd.dma_gather(
                    out_ap=hTc[:, ch], in_ap=hp[:],
                    idxs_ap=idxt[:, o // 16:(o + 512) // 16],
                    num_idxs=512, num_idxs_reg=512, elem_size=H, transpose=True,
                )
            for ch in range(nch):
                ps = xpsp.tile([128, 512], F32, tag="x")
                for k in range(KCH):
                    nc.tensor.matmul(
                        ps[:], Wt[:, k, :], hTc[:, ch, k, :],
                        start=(k == 0), stop=(k == KCH - 1),
                    )
                nc.scalar.activation(
                    Xc[:, ch * 512:(ch + 1) * 512], ps[:],
                    AF.Identity, bias=bcol[:, 0:1], scale=1.0,
                )

        if stop == "encode":
            fsb = bigp.tile([B, NP_LOC], F32)
            nc.vector.tensor_copy(fsb[:], X[:B, :NP_LOC])
            nc.sync.dma_start(outd[:], fsb[:])
            return

        # ---- norms over token axis (per (b, d)), from the FULL encode --
        nsqp = bigp.tile([128, NP_LOC], F32)
        nsqq = bigp.tile([128, B], F32)
        with tc.tile_pool(name="sqd", bufs=1) as sqdp:
            sq = sqdp.tile([128, PTOK + QTOK], BF)
            nc.vector.tensor_mul(sq[:, :PTOK], X[:, :PTOK], X[:, :PTOK])
            nc.vector.tensor_mul(sq[:, PTOK:], X[:, PTOK:], X[:, PTOK:])
            nc.vector.reduce_sum(
                nsqp[:], sq[:, :PTOK].rearrange("p (c l) -> p c l", c=NP_LOC),
                axis=AX.X)
            nc.vector.reduce_sum(
                nsqq[:], sq[:, PTOK:].rearrange("p (b l) -> p b l", b=B),
                axis=AX.X)

        if stop == "norms":
            fsb = bigp.tile([B, NP_LOC], F32)
            nc.vector.tensor_copy(fsb[:], nsqp[:B, :])
            nc.sync.dma_start(outd[:], fsb[:])
            return

        # scale = 1 / max(sqrt(nsq), 1e-12)
        sclp = bigp.tile([128, NP_LOC], F32)
        sclq = bigp.tile([128, B], F32)
        nc.scalar.sqrt(sclp[:], nsqp[:])
        nc.vector.tensor_scalar_max(sclp[:], sclp[:], 1e-12)
        nc.vector.reciprocal(sclp[:], sclp[:])
        nc.scalar.sqrt(sclq[:], nsqq[:])
        nc.vector.tensor_scalar_max(sclq[:], sclq[:], 1e-12)
        nc.vector.reciprocal(sclq[:], sclq[:])

        # ---- scale (compacted passages need no mask) -------------------
        P2 = bigp.tile([128, ni], BF)
        Q2 = bigp.tile([128, QTOK], BF)
        for c in range(NP_LOC):
            nc.vector.tensor_scalar_mul(
                P2[:, c * n_pad:(c + 1) * n_pad],
                Xc[:, c * n_pad:(c + 1) * n_pad],
                sclp[:, c:c + 1],
            )
        for b_ in range(B):
            nc.scalar.mul(
                Q2[:, b_ * LQ:(b_ + 1) * LQ],
                X[:, PTOK + b_ * LQ:PTOK + (b_ + 1) * LQ],
                sclq[:, b_:b_ + 1],
            )

        if stop == "scale":
            fsb = bigp.tile([B, NP_LOC], F32)
            nc.vector.tensor_copy(fsb[:], P2[:B, :NP_LOC])
            nc.sync.dma_start(outd[:], fsb[:])
            return

        # ---- scores: per query-tile, matmul against compacted passages,
        # ---- MaxSim via free-axis reduce_max ---------------------------
        # psum tiles keep a 256-column (1KB, bank-aligned) stride per
        # passage; only the first n_pad columns are written and reduced.
        NQT = QTOK // 128                               # 16 query tiles
        Mx = bigp.tile([128, NQT * NP_LOC], F32)        # [q, qt*16+c] maxima
        tpb = max(1, 512 // n_pad)                      # score slots per bank
        gsz0 = 2 * tpb                                  # passages per psum tile
        groups = []
        c0 = 0
        while c0 < NP_LOC:
            groups.append((c0, min(gsz0, NP_LOC - c0)))
            c0 += gsz0
        with tc.tile_pool(name="sps", bufs=3, space="PSUM") as spsp:
            for qt in range(NQT):
                for c0, gsz in groups:
                    ps = spsp.tile([128, 2, 512], F32, tag="s")
                    for i in range(gsz):
                        c = c0 + i
                        nc.tensor.matmul(
                            ps[:, i // tpb, (i % tpb) * n_pad:
                               (i % tpb) * n_pad + n_pad],
                            Q2[:, qt * 128:(qt + 1) * 128],
                            P2[:, c * n_pad:(c + 1) * n_pad],
                            start=True, stop=True,
                        )
                    nb = -(-gsz // tpb)                 # banks used this group
                    red_in = ps[:, :nb, :tpb * n_pad].rearrange(
                        "p b (s n) -> p b s n", n=n_pad)
                    nc.vector.reduce_max(
                        Mx[:, qt * NP_LOC + c0: qt * NP_LOC + c0 + gsz],
                        red_in[:, :, :, :] if gsz == nb * tpb else None,
                        axis=AX.X,
                    ) if gsz == nb * tpb else None
                    if gsz != nb * tpb:
                        full = (gsz // tpb) * tpb
                        if full:
                            nc.vector.reduce_max(
                                Mx[:, qt * NP_LOC + c0: qt * NP_LOC + c0 + full],
                                red_in[:, :gsz // tpb], axis=AX.X)
                        rem = gsz - full
                        nc.vector.reduce_max(
                            Mx[:, qt * NP_LOC + c0 + full:
                               qt * NP_LOC + c0 + gsz],
                            red_in[:, gsz // tpb:gsz // tpb + 1, :rem], axis=AX.X)

        if stop == "maxred":
            fsb = bigp.tile([B, NP_LOC], F32)
            nc.vector.tensor_copy(fsb[:], Mx[:B, :NP_LOC])
            nc.sync.dma_start(outd[:], fsb[:])
            return

        # ---- sum over the 32 queries of each batch via PE --------------
        A = bigp.tile([128, 4], F32)                    # block-ones
        nc.vector.memset(A[:], 0.0)
        for i in range(4):
            nc.vector.memset(A[32 * i:32 * (i + 1), i:i + 1], 1.0)
        fin = finp.tile([4, NQT * NP_LOC], F32)
        for qt in range(NQT):
            nc.tensor.matmul(
                fin[:, qt * NP_LOC:(qt + 1) * NP_LOC],
                A[:], Mx[:, qt * NP_LOC:(qt + 1) * NP_LOC],
                start=True, stop=True,
            )
        # fin[b4, qt*16+c] -> out[qt*4+b4, c]
        fsb = bigp.tile([4, NQT * NP_LOC], F32)
        nc.vector.tensor_copy(fsb[:], fin[:])
        nc.sync.dma_start(
            outd.rearrange("(t b4) c -> b4 t c", b4=4),
            fsb[:].rearrange("p (t c) -> p t c", t=NQT),
        )


def _prep_inputs(query_hidden, positive_hidden, negative_hidden, W, b,
                 positive_filter_mask, negative_filter_mask):
    """Returns (in_maps, n_pad)."""
    hq = np.ascontiguousarray(
        np.asarray(query_hidden, np.float32).reshape(QTOK, H)).astype(BF16)
    Wb = np.asarray(W, np.float32).astype(BF16)
    bcol = np.ascontiguousarray(np.asarray(b, np.float32).reshape(D, 1))
    pos = np.asarray(positive_hidden, np.float32)
    neg = np.asarray(negative_hidden, np.float32)
    pmask = np.asarray(positive_filter_mask).astype(bool)
    nmask = np.asarray(negative_filter_mask).astype(bool)

    core_masks = []
    n_max = 1
    for j in range(NCORES):
        sl = slice(j * CPC, (j + 1) * CPC)
        m = np.concatenate([pmask[sl], nmask[sl]], axis=0)   # [16, 256]
        core_masks.append(m)
        n_max = max(n_max, int(m.sum(axis=1).max()))
    n_pad = -(-n_max // 32) * 32          # mult of 32 so ni % 512 == 0
    ni = NP_LOC * n_pad

    in_maps = []
    for j in range(NCORES):
        sl = slice(j * CPC, (j + 1) * CPC)
        hp = np.concatenate(
            [pos[sl].reshape(CPC * LP, H), neg[sl].reshape(CPC * LP, H)], axis=0
        ).astype(BF16)
        m = core_masks[j]
        idx = np.zeros((NP_LOC, n_pad), np.int16)
        for c in range(NP_LOC):
            pos_c = np.nonzero(m[c])[0]
            idx[c, :len(pos_c)] = pos_c + c * LP
            idx[c, len(pos_c):] = c * LP                      # dup token 0
        flat = idx.reshape(ni)
        wrapped = np.tile(flat.reshape(ni // 16, 16).T, (8, 1)).copy()
        in_maps.append({
            "hq": hq, "hp": hp, "W": Wb, "b": bcol, "pidx": wrapped,
        })
    return in_maps, n_pad


def _assemble(results):
    out = np.zeros((B, 2 * B), np.float32)
    for j in range(NCORES):
        o = np.asarray(results[j]["out"], np.float32)      # [64, 16]
        out[:, j * CPC:(j + 1) * CPC] = o[:, :CPC]
        out[:, B + j * CPC:B + (j + 1) * CPC] = o[:, CPC:]
    return out


def kernel(query_hidden, positive_hidden, negative_hidden, W, b,
           positive_filter_mask, negative_filter_mask):
    in_maps, n_pad = _prep_inputs(query_hidden, positive_hidden, negative_hidden,
                                  W, b, positive_filter_mask, negative_filter_mask)
    nc = build_program(n_pad)
    res = run_bass_kernel_spmd(nc, in_maps, list(range(NCORES)))
    return _assemble(res.results)
